# revision 1
# baseline (speedup 1.0000x reference)
"""Trainium2 Bass kernel for CoAttention — bf16, DVE-scored, DMA-tuned.

Math (per batch b):
    s_sum = sum_q(sentence)                          [D]
    w     = s_sum @ (Wq.T @ Wk) + Lq*(bq @ Wk)       [D]   (weight product fused
                                                            on host; bk dropped:
                                                            softmax shift-invariant)
    s_k   = comment[k] . w                           [Lk]
    p     = exp(s - max s);  l = sum p
    ctx   = (p @ comment) / l                        [D]
    out   = ctx @ Wv.T + bv                          [D]

Sharding: data-parallel over batch, 4 batches per core, weights replicated.
Big tensors ship as bf16 (f32 accumulation everywhere; softmax here is
effectively one-hot — min top-2 score gap 5.5 vs score std 269 — measured
end-to-end rel err ~5e-3 vs tol 2e-2).

Bottleneck analysis (HW-measured): the 64 fused mul+reduce score ops on DVE
cost ~1.13 us each (no 2x bf16 mode for accumulating DVE ops) and the 18
MB/core bf16 working set streams at ~220-320 GB/s, so DVE compute and DMA are
nearly co-critical (~76 us vs ~60-83 us). Score tiles and p-tiles are
double-buffered per batch so batch b+1's producers never stall on batch b's
cross-engine readers; 1/l is folded into the PE ctx-transpose as a
scaled-identity matmul (no extra DVE pass). Offloading score tiles to GpSimd
(ISA-rejected on Pool) or to an ACT copy-accum lane (measured slower: engine
ping-pong) did not help; all scores stay on DVE.
"""

import numpy as np

B, LQ, LK, D = 32, 512, 2048, 768
NCORES = 8
BPC = B // NCORES      # 4 batches per core
TQ = LQ // 128         # 4 q-subtiles per partition
TK = LK // 128         # 16 k-subtiles per partition
HALF = TK // 2         # 8 k-subtiles per comment DMA tile
DC = D // 128          # 6 d-chunks
ACT_TILES = [0, 0, 0, 0]   # ACT-reduce score lane disabled (HW-measured slower)
PERM = False               # host-permuted partition-major DRAM layout

_cache = {}


def _split_multi_waits(nc):
    """This walrus build allows only ONE sync-wait command per instruction.
    Tile emits several when an instruction depends on multiple procs. Hoist
    the extras onto same-engine NoOps inserted immediately before (the engine
    queue is FIFO, so the waits execute in order — semantically identical)."""
    import bass_rust
    from concourse import mybir

    n_split = 0
    for f in nc.m.functions:
        for bb in f.blocks:
            out = []
            for inst in bb.instructions:
                si = inst.sync_info
                waits = list(si.on_wait or []) if si else []
                if len(waits) > 1:
                    for i, w in enumerate(waits[:-1]):
                        nop = mybir.InstNoOp(name=f"{inst.name}-ws{i}")
                        nop.engine = inst.engine
                        nop.bass_nofuse = True
                        nop.sync_info = bass_rust.SyncInfo(
                            on_wait=[w], on_update=[]
                        )
                        out.append(nop)
                        n_split += 1
                    si.on_wait = waits[-1:]
                out.append(inst)
            bb.instructions[:] = out
    return n_split


def build_program(split_waits=True, reps=1, act_tiles=None, comm_halves=False,
                  weights_act_ring=False, wqk_act=True, perm=None,
                  hoist=True, psum3=True, deep_bufs=True):
    if perm is None:
        perm = PERM
    if act_tiles is None:
        act_tiles = ACT_TILES
    nab = 3 if psum3 else 2   # A-tag bufs; B stays 2 (bank budget)
    ndb = 3 if deep_bufs else 2  # per-batch pipeline tile bufs
    import contextlib

    import concourse.bass as bass
    import concourse.tile as tile
    from concourse import masks, mybir

    f32 = mybir.dt.float32
    bf16 = mybir.dt.bfloat16
    Alu = mybir.AluOpType
    Act = mybir.ActivationFunctionType
    Axis = mybir.AxisListType

    nc = bass.Bass()
    if perm:
        # host ships partition-major arrays: per-partition DRAM runs are
        # 49-98 KB contiguous instead of 24 KB (longer SDMA descriptors)
        sent = nc.declare_dram_parameter("sent", [128, BPC, TQ, D], bf16,
                                         isOutput=False)
        comm = nc.declare_dram_parameter("comm", [128, BPC, TK, D], bf16,
                                         isOutput=False)
    else:
        sent = nc.declare_dram_parameter("sent", [BPC, LQ, D], bf16,
                                         isOutput=False)
        comm = nc.declare_dram_parameter("comm", [BPC, LK, D], bf16,
                                         isOutput=False)
    wqk = nc.declare_dram_parameter("wqk", [D, D], bf16, isOutput=False)
    wvt = nc.declare_dram_parameter("wvt", [D, D], bf16, isOutput=False)
    bqk = nc.declare_dram_parameter("bqk", [D], f32, isOutput=False)
    bv = nc.declare_dram_parameter("bv", [D], f32, isOutput=False)
    out = nc.declare_dram_parameter("out", [BPC, D], f32, isOutput=True)

    # q = p*TQ + t, k = p*TK + t: per-(partition, batch) contiguous DRAM runs
    if perm:
        sent_r = sent[:]
        comm_r = comm[:]
    else:
        sent_r = sent.rearrange("b (p t) d -> p b t d", p=128)
        comm_r = comm.rearrange("b (p t) d -> p b t d", p=128)
    wqk_r = wqk.rearrange("(c p) e -> p c e", p=128)         # [128,DC,D]
    wvt_r = wvt.rearrange("(c p) e -> p c e", p=128)

    with tile.TileContext(nc) as tc:
      with tc.tile_pool(name="consts", bufs=1) as consts:
        cbox = {}

        def emit_consts():
            ident = consts.tile([128, 128], f32, tag="ident")
            masks.make_identity(nc, ident[:])
            ones_col_bf = consts.tile([128, 1], bf16, tag="ocb")
            nc.vector.memset(ones_col_bf[:], 1.0)
            ones_col_f = consts.tile([128, 1], f32, tag="ocf")
            nc.vector.memset(ones_col_f[:], 1.0)
            ones_row_f = consts.tile([1, 128], f32, tag="orf")
            nc.vector.memset(ones_row_f[:], 1.0)
            dummy = consts.tile([1, 1], f32, tag="dum")
            nc.vector.memset(dummy[:], 0.0)
            nc.scalar.activation(dummy[:], dummy[:], Act.Exp)
            cbox.update(ident=ident, ones_col_bf=ones_col_bf,
                        ones_col_f=ones_col_f, ones_row_f=ones_row_f)

        if hoist:
            emit_consts()
        rep_loop = tc.For_i(0, reps, 1) if reps > 1 else contextlib.nullcontext()
        with rep_loop:
          with (
            tc.tile_pool(name="big", bufs=1) as big,
            tc.tile_pool(name="commp", bufs=1) as commp,
            tc.tile_pool(name="rows", bufs=1) as rows,
            tc.tile_pool(name="smalls", bufs=2) as smalls,
            tc.tile_pool(name="dramp", bufs=1, space="DRAM") as dramp,
            tc.tile_pool(name="ps", bufs=1, space="PSUM") as ps,
          ):
            # ---------------- constants (no DMA) ----------------
            if not hoist:
                emit_consts()
            ident = cbox["ident"]
            ones_col_bf = cbox["ones_col_bf"]
            ones_col_f = cbox["ones_col_f"]
            ones_row_f = cbox["ones_row_f"]

            # ---------------- DMA issue order on the SP ring -------------
            # sentence (phase-0 critical) -> wqk -> comment -> wvt (end-only)
            # few, large transfers: per-DMA overhead is ~3 us on this stack
            # (HW-probed: 8x1.57MB = 83 us vs 4x3.15MB+3 = 57 us for the same
            # bytes), so the whole shard moves in 7 DMAs on the SP ring.
            sent_all = big.tile([128, BPC, TQ, D], bf16)
            nc.sync.dma_start(out=sent_all[:], in_=sent_r[:])
            sent_sb = [sent_all[:, b] for b in range(BPC)]
            weng = nc.scalar if weights_act_ring else nc.sync
            wqk_eng = nc.scalar if (weights_act_ring or wqk_act) else nc.sync
            wqk_sb = big.tile([128, DC, D], bf16)
            wqk_eng.dma_start(out=wqk_sb[:], in_=wqk_r[:])
            comm_tiles = {}
            if comm_halves:
                for b in range(BPC):
                    for h in range(2):
                        th = commp.tile([128, HALF, D], bf16, tag=f"c{b}{h}")
                        nc.sync.dma_start(
                            out=th[:],
                            in_=comm_r[:, b, h * HALF : (h + 1) * HALF, :])
                        comm_tiles[(b, h)] = th
            elif perm:
                # pair-batch DMAs: 49 KB contiguous per partition, and
                # iteration i+1's first pair only waits on ctx of batch 1
                # (mid-iteration), preserving cross-iteration overlap
                for pr in range(2):
                    t = commp.tile([128, 2, TK, D], bf16, tag=f"cp{pr}")
                    nc.sync.dma_start(out=t[:],
                                      in_=comm_r[:, 2 * pr : 2 * pr + 2])
                    comm_tiles[pr] = t
            else:
                for b in range(BPC):
                    t = commp.tile([128, TK, D], bf16, tag=f"c{b}")
                    nc.sync.dma_start(out=t[:], in_=comm_r[:, b, :, :])
                    comm_tiles[b] = t
            wvt_sb = big.tile([128, DC, D], bf16)
            weng.dma_start(out=wvt_sb[:], in_=wvt_r[:])

            # small loads on the ACT ring (bypass the bulk FIFO)
            bqk_row = rows.tile([1, D], f32)
            nc.scalar.dma_start(out=bqk_row[:], in_=bqk[None, :])
            bv_row = rows.tile([1, D], f32)
            nc.scalar.dma_start(out=bv_row[:], in_=bv[None, :])

            # ---------------- phase 0: s_sum, w, wb broadcast -----------
            s_flat = rows.tile([1, BPC, D], f32)
            for b in range(BPC):
                ssa = ps.tile([1, 512], f32, tag="A", bufs=nab)
                ssb = ps.tile([1, 256], f32, tag="B", bufs=2)
                for t in range(TQ):
                    nc.tensor.matmul(ssa[:], ones_col_bf[:],
                                     sent_sb[b][:, t, 0:512],
                                     start=(t == 0), stop=(t == TQ - 1))
                for t in range(TQ):
                    nc.tensor.matmul(ssb[:], ones_col_bf[:],
                                     sent_sb[b][:, t, 512:768],
                                     start=(t == 0), stop=(t == TQ - 1))
                nc.scalar.copy(s_flat[0:1, b, 0:512], ssa[:])
                nc.scalar.copy(s_flat[0:1, b, 512:768], ssb[:])

            # ssT chunks [128, DC, BPC] bf16 (4 transposes + 1 copy per chunk)
            ssT = smalls.tile([128, DC, BPC], bf16, tag="ssT", bufs=1)
            for c in range(DC):
                pt4 = ps.tile([128, BPC], f32, tag="C", bufs=3)
                for b in range(BPC):
                    nc.tensor.transpose(
                        pt4[:, b : b + 1],
                        s_flat[0:1, b, c * 128 : (c + 1) * 128],
                        ident[0:1, 0:1],
                    )
                nc.scalar.copy(ssT[:, c, :], pt4[:])

            # w rows [BPC, D] = ssT.T @ Wqk + bqk  (bqk pre-scaled by Lq)
            w_sb = rows.tile([BPC, D], bf16)
            pwa = ps.tile([BPC, 512], f32, tag="A", bufs=nab)
            pwb = ps.tile([BPC, 256], f32, tag="B", bufs=2)
            for c in range(DC):
                nc.tensor.matmul(pwa[:], ssT[:, c, :], wqk_sb[:, c, 0:512],
                                 start=(c == 0), stop=False)
            nc.tensor.matmul(pwa[:], ones_row_f[0:1, 0:BPC],
                             bqk_row[0:1, 0:512], start=False, stop=True)
            for c in range(DC):
                nc.tensor.matmul(pwb[:], ssT[:, c, :], wqk_sb[:, c, 512:768],
                                 start=(c == 0), stop=False)
            nc.tensor.matmul(pwb[:], ones_row_f[0:1, 0:BPC],
                             bqk_row[0:1, 512:768], start=False, stop=True)
            nc.scalar.copy(w_sb[:, 0:512], pwa[:])
            nc.scalar.copy(w_sb[:, 512:768], pwb[:])

            # DRAM bounce + single stride-0 broadcast DMA (ACT ring)
            import concourse.bass as bass_mod
            w_dram = dramp.tile([BPC, D], bf16)
            nc.scalar.dma_start(out=w_dram[:], in_=w_sb[:])
            wb_all = smalls.tile([128, BPC, D], bf16, tag="wb", bufs=2)
            src = w_dram[:, :]
            src_bcast = bass_mod.AP(
                tensor=src.tensor, offset=src.offset,
                ap=[[0, 128]] + list(src.ap),
            )
            nc.scalar.dma_start(out=wb_all[:], in_=src_bcast)

            # ---------------- main loop over batches ----------------
            # scores: ACT-lane tiles get a DVE product (2x bf16 tensor_tensor)
            # reduced by an ACT Copy+accum; the rest run fused on DVE. Lanes
            # are balanced at ~60 us each (HW-measured per-op rates).
            na_max = max(max(act_tiles), 1)
            nd_max = TK - min(act_tiles)
            ttr_d = smalls.tile([128, D], bf16, tag="ttrd", bufs=1)
            ttr_a = smalls.tile([128, D], bf16, tag="ttra", bufs=1)
            ctxT = smalls.tile([128, DC, BPC], bf16, tag="ctxT", bufs=1)

            def slab(b, t):
                if comm_halves:
                    return comm_tiles[(b, t // HALF)], t % HALF
                if perm:
                    return comm_tiles[b // 2][:, b % 2], t
                return comm_tiles[b], t

            for b in range(BPC):
                na = act_tiles[b]
                # per-batch score/p tiles, double-buffered so batch b+1's
                # producers never wait on batch b's cross-engine readers
                s_cols_a = smalls.tile([128, na_max], f32, tag="sca", bufs=ndb)
                s_cols_d = smalls.tile([128, nd_max], f32, tag="scd", bufs=ndb)
                p_cols = smalls.tile([128, TK], bf16, tag="pcols", bufs=ndb)
                invl1 = smalls.tile([1, 1], f32, tag="invl1", bufs=ndb)
                # ACT lane: DVE product -> ACT reduce (double-buffered prods)
                for j in range(na):
                    ct, tt = slab(b, j)
                    prod = smalls.tile([128, D], bf16, tag="prod", bufs=3)
                    nc.vector.tensor_tensor(
                        out=prod[:], in0=ct[:, tt, :], in1=wb_all[:, b, :],
                        op=Alu.mult,
                    )
                    nc.scalar.activation(
                        ttr_a[:], prod[:], Act.Copy,
                        accum_out=s_cols_a[:, j : j + 1],
                    )
                # DVE lane: fused mul+reduce
                for j in range(na, TK):
                    ct, tt = slab(b, j)
                    nc.vector.scalar_tensor_tensor(
                        out=ttr_d[:], in0=ct[:, tt, :], scalar=1.0,
                        in1=wb_all[:, b, :], op0=Alu.mult, op1=Alu.mult,
                        accum_out=s_cols_d[:, j - na : j - na + 1],
                    )

                # softmax (full batch): row maxes, cross-partition max via PE
                rm_d = smalls.tile([128, 1], f32, tag="rmd", bufs=2)
                nc.vector.tensor_reduce(
                    out=rm_d[:], in_=s_cols_d[:, 0 : TK - na], axis=Axis.X,
                    op=Alu.max,
                )
                if na > 0:
                    rm_p = smalls.tile([128, 1], f32, tag="rmp", bufs=2)
                    nc.vector.tensor_reduce(
                        out=rm_p[:], in_=s_cols_a[:, 0:na], axis=Axis.X,
                        op=Alu.max,
                    )
                    rmx = smalls.tile([128, 1], f32, tag="rmx", bufs=2)
                    nc.vector.tensor_tensor(out=rmx[:], in0=rm_d[:],
                                            in1=rm_p[:], op=Alu.max)
                else:
                    rmx = rm_d
                prm = ps.tile([1, 128], f32, tag="C", bufs=3)
                nc.tensor.transpose(prm[:], rmx[:], ident[:])
                rm_row = smalls.tile([1, 128], f32, tag="rmrow", bufs=2)
                nc.scalar.copy(rm_row[:], prm[:])
                M_sb = smalls.tile([1, 1], f32, tag="M", bufs=2)
                nc.vector.tensor_reduce(out=M_sb[:], in_=rm_row[:], axis=Axis.X,
                                        op=Alu.max)
                pnm = ps.tile([128, 1], f32, tag="C", bufs=3)
                nc.tensor.matmul(pnm[:], ones_row_f[:], M_sb[:])
                nm = smalls.tile([128, 1], f32, tag="nm", bufs=2)
                nc.scalar.activation(nm[:], pnm[:], Act.Copy, scale=-1.0)
                rowsums = []
                if na > 0:
                    rs_d = smalls.tile([128, 1], f32, tag="rsd", bufs=2)
                    nc.scalar.activation(
                        p_cols[:, 0:na], s_cols_a[:, 0:na], Act.Exp,
                        bias=nm[:], scale=1.0, accum_out=rs_d[:],
                    )
                    rowsums.append(rs_d)
                rs_p = smalls.tile([128, 1], f32, tag="rsp", bufs=2)
                nc.scalar.activation(
                    p_cols[:, na:TK], s_cols_d[:, 0 : TK - na], Act.Exp,
                    bias=nm[:], scale=1.0, accum_out=rs_p[:],
                )
                rowsums.append(rs_p)
                pl = ps.tile([1, 1], f32, tag="C", bufs=3)
                for i, rs in enumerate(rowsums):
                    nc.tensor.matmul(pl[:], rs[:], ones_col_f[:],
                                     start=(i == 0),
                                     stop=(i == len(rowsums) - 1))
                l_sb = smalls.tile([1, 1], f32, tag="lsb", bufs=2)
                nc.scalar.copy(l_sb[:], pl[:])
                nc.vector.reciprocal(invl1[:], l_sb[:])

                # ctx accumulation on PE: ctx = p @ comment (unnormalized)
                ca = ps.tile([1, 512], f32, tag="A", bufs=nab)
                cb = ps.tile([1, 256], f32, tag="B", bufs=2)
                for t in range(TK):
                    ct, tt = slab(b, t)
                    pcol = p_cols[:, t : t + 1]
                    nc.tensor.matmul(ca[:], pcol, ct[:, tt, 0:512],
                                     start=(t == 0), stop=(t == TK - 1))
                    nc.tensor.matmul(cb[:], pcol, ct[:, tt, 512:768],
                                     start=(t == 0), stop=(t == TK - 1))
                ctxn = rows.tile([1, D], f32, tag="ctxn", bufs=2)
                nc.scalar.copy(ctxn[0:1, 0:512], ca[:])
                nc.scalar.copy(ctxn[0:1, 512:768], cb[:])

                # transpose ctx chunks, folding the 1/l scale into the PE
                # "transpose" (matmul against a 1x1 scaled identity)
                pt6 = ps.tile([128, DC], f32, tag="C", bufs=3)
                for c in range(DC):
                    nc.tensor.transpose(
                        pt6[:, c : c + 1],
                        ctxn[0:1, c * 128 : (c + 1) * 128],
                        invl1[0:1, 0:1],
                    )
                nc.scalar.copy(ctxT[:, :, b], pt6[:])

            # ---------------- final projection ----------------
            poa = ps.tile([BPC, 512], f32, tag="A", bufs=nab)
            pob = ps.tile([BPC, 256], f32, tag="B", bufs=2)
            for c in range(DC):
                nc.tensor.matmul(poa[:], ctxT[:, c, :], wvt_sb[:, c, 0:512],
                                 start=(c == 0), stop=False)
                nc.tensor.matmul(pob[:], ctxT[:, c, :], wvt_sb[:, c, 512:768],
                                 start=(c == 0), stop=False)
            nc.tensor.matmul(poa[:], ones_row_f[0:1, 0:BPC],
                             bv_row[0:1, 0:512], start=False, stop=True)
            nc.tensor.matmul(pob[:], ones_row_f[0:1, 0:BPC],
                             bv_row[0:1, 512:768], start=False, stop=True)
            out_sb = rows.tile([BPC, D], f32)
            nc.scalar.copy(out_sb[:, 0:512], poa[:])
            nc.scalar.copy(out_sb[:, 512:768], pob[:])
            nc.scalar.dma_start(out=out[:], in_=out_sb[:])

    if split_waits:
        _split_multi_waits(nc)
    return nc


def _get_program():
    if "nc" not in _cache:
        _cache["nc"] = build_program()
    return _cache["nc"]


def _make_in_maps(sentence_rep, comment_rep, Wq, bq, Wk, bk, Wv, bv,
                  perm=None):
    import ml_dtypes

    if perm is None:
        perm = PERM
    del bk  # softmax is shift-invariant: the bk term cancels exactly
    bf = ml_dtypes.bfloat16
    Wq = np.asarray(Wq, np.float32)
    Wk = np.asarray(Wk, np.float32)
    Wv = np.asarray(Wv, np.float32)
    wqk = np.ascontiguousarray((Wq.T @ Wk).astype(bf))
    bqk = np.ascontiguousarray(
        (float(LQ) * (np.asarray(bq, np.float32) @ Wk)).astype(np.float32))
    wvt = np.ascontiguousarray(Wv.T.astype(bf))
    bv_ = np.ascontiguousarray(np.asarray(bv, dtype=np.float32))
    sent = np.asarray(sentence_rep, np.float32).astype(bf)
    comm = np.asarray(comment_rep, np.float32).astype(bf)
    in_maps = []
    for c in range(NCORES):
        sl = slice(c * BPC, (c + 1) * BPC)
        sent_s, comm_s = sent[sl], comm[sl]
        if perm:
            # [b, (p t), d] -> [p, b, t, d], partition-major contiguous
            sent_s = sent_s.reshape(BPC, 128, TQ, D).transpose(1, 0, 2, 3)
            comm_s = comm_s.reshape(BPC, 128, TK, D).transpose(1, 0, 2, 3)
        in_maps.append({
            "sent": np.ascontiguousarray(sent_s),
            "comm": np.ascontiguousarray(comm_s),
            "wqk": wqk, "wvt": wvt, "bqk": bqk, "bv": bv_,
        })
    return in_maps


def run(inputs, trace=False, **kwargs):
    from concourse.bass_utils import run_bass_kernel_spmd

    nc = _get_program()
    in_maps = _make_in_maps(**inputs)
    res = run_bass_kernel_spmd(
        nc, in_maps, list(range(NCORES)), trace=trace, **kwargs
    )
    out = np.concatenate([res.results[c]["out"] for c in range(NCORES)], axis=0)
    return out.astype(np.float32), res


def kernel(**inputs) -> np.ndarray:
    out, _ = run(inputs)
    return out

